# revision 3
# baseline (speedup 1.0000x reference)
"""Trainium2 Bass kernel: BoxSeg DynamicMaskHead compute_pairwise_term.

For each instance n and each of the 8 non-center taps (dy, dx) of a 3x3
dilation-2 stencil:

    out[n, t, h, w] = sp(x[h,w]) + sp(x[h+dy,w+dx]) - sp(x[h,w] + x[h+dy,w+dx])

with sp = softplus, computed as E = exp(x), L = ln(E + 1) and the tap term
ln(1 + E_c * E_y).  Mirror symmetry out[(dy,dx)][h,w] == out[(-dy,-dx)][h+dy,
w+dx] means only 4 of the 8 tap fields are unique; the device computes each
exactly once and dumps it linearly to DRAM, and the host assembles the full
[N, 8, H, W] tensor (mirror placement, boundary zeros, halo stripping, f32
upcast) -- pure data movement, no arithmetic.  That halves HBM write traffic
vs storing all 8 taps and lets every store be a full-width linear DMA
(4160B-per-partition descriptors at line rate instead of 512B row pieces).

Row-pair layout: partition p holds image rows {2p, 2p+1} (j = r % 2), so the
whole 256-row frame fits one tile and the dy=-2 row shift is a single
partition shift, materialized two ways: for E by an SBUF->SBUF partition-
shift DMA (E2), and for the softplus sum Lsum = L_c + L_y on the
TensorEngine by accumulating identity matmuls (eye and a k=1-shifted eye).
P = E_c * E_y runs on DVE (one quarter on GpSimd); ln(1+P) on ACT; the final
(ln_tap * -1) + Lsum on DVE reading PSUM.

Everything is fp16 (input downcast on host; P is bf16 since e^{a+b} can
reach ~6e4, too close to fp16 max).  ACT is the bottleneck engine
(~5 softplus-class evals per pixel are irreducible), so: the scalar engine
issues no DMAs, activations are merged into few large calls (the 352-cycle
per-call startup matters), and a dummy activation at t=0 overlaps the one
~2.7us ACT table load with the first input DMA.  Groups of G=4 instances
amortize fixed costs; 2 groups software-pipeline against each other.

Boundary handling is free: out-of-range rows/cols of each quarter (partition
0 of the row-shifted quarters, halo columns) compute garbage that the host
never reads -- it zero-fills those regions during assembly.

Sharding: data-parallel over N=64 -> 8 instances per core on 8 NeuronCores.
Self-contained: shapes hardcoded.
"""

import os

import numpy as np

N_CORES = 8
N_FULL = 64
N_PER = N_FULL // N_CORES  # 8 instances per core
H = W = 256
G = 4  # instances per group; 2 groups per core
NGRP = N_PER // G

# SBUF free-dim layouts (elements per partition); partition p = rows
# {2p, 2p+1}, j = r % 2, cc = image col + 2 (2-col halo each side).
# X: [g(G), jc(512)]
# E/L/E2: [g, j(2), cc(260)]
XG, XJ = 520, 260
XF = G * 520
# P/ln_t: [q(4), g, j, c(256)]
PQ, PG, PJ = G * 512, 512, 256
PF = 4 * G * 512
# o: [g, q, j, cc(260)]
OG, OQ, OJ = 2080, 520, 260
OF = G * 2080

XN = H * W  # x[n, r, c] instance stride
ON = 128 * 2080  # out[n, p, e] instance stride

_CACHE = {}


def _force_combined_act_table():
    """Make the table-load inserter see only the one set containing both Exp
    and Ln (all other sets emptied, positions preserved so act_func_set_id
    still indexes the real act_info.json).  Without this the inserter
    alternates between the exp- and ln-anchored sets: one 1.28us
    ACT_TABLE_LOAD per Exp<->Ln transition, which dominates the runtime."""
    import concourse.bacc as bacc
    import concourse.hw_specs as hw_specs
    import concourse.mybir as mybir

    real = dict(hw_specs.get_activation_tables("gen3"))
    target = None
    for name, fns in real.items():
        if (
            mybir.ActivationFunctionType.Exp in fns
            and mybir.ActivationFunctionType.Ln in fns
        ):
            target = name
            break
    assert target is not None, "no act table set with both Exp and Ln"
    patched = {
        name: (fns if name == target else set()) for name, fns in real.items()
    }
    bacc.get_activation_tables = lambda arch: patched
    hw_specs.get_activation_tables = lambda arch: patched


def _build_program():
    import concourse.bacc as bacc
    import concourse.mybir as mybir
    from concourse import tile

    if not os.environ.get("KERNEL_NO_ACT_PATCH"):
        _force_combined_act_table()

    f32 = mybir.dt.float32
    f16 = mybir.dt.float16
    bf16 = mybir.dt.bfloat16
    EXP = mybir.ActivationFunctionType.Exp
    LN = mybir.ActivationFunctionType.Ln
    ADD = mybir.AluOpType.add
    MULT = mybir.AluOpType.mult

    def mk(base, dims, off=0):
        """Rebuild the free dims of an AP: keep base's partition dim (ap[0]),
        replace the rest with `dims` ([step, count] in elements), and advance
        the offset by `off` elements."""
        c = base.copy()
        c.ap = mybir.VecI64Pair([list(c.ap[0])] + [list(d) for d in dims])
        c.offset = c.offset + off
        return c

    def mkd(base, dims, off=0):
        """Same for DRAM APs (no partition dim to preserve)."""
        c = base.copy()
        c.ap = mybir.VecI64Pair([list(d) for d in dims])
        c.offset = c.offset + off
        return c

    nc = bacc.Bacc(
        "TRN2",
        target_bir_lowering=False,
        debug=False,
        enable_asserts=False,
        num_devices=N_CORES,
    )
    x = nc.dram_tensor("x", [N_PER, H, W], f16, kind="ExternalInput").ap()
    out = nc.dram_tensor("out", [N_PER, 128, 2080], f16, kind="ExternalOutput").ap()
    eye = nc.dram_tensor("eye", [128, 128], f16, kind="ExternalInput").ap()
    eye_s1 = nc.dram_tensor("eye_s1", [128, 128], f16, kind="ExternalInput").ap()

    with tile.TileContext(nc) as tc:
        with (
            tc.tile_pool(name="cst", bufs=1) as cst,
            tc.tile_pool(name="io", bufs=2) as iop,
            tc.tile_pool(name="wk", bufs=2) as wp,
            tc.tile_pool(name="ps", bufs=4, space="PSUM") as psp,
        ):
            # dummy activation with no data deps: pulls the one ACT table
            # load (~2.7us) to t=0, under the first input DMA
            dummy = cst.tile([128, 16], f16)
            nc.vector.memset(dummy[:, :], 0.0)
            nc.scalar.activation(dummy[:, :], dummy[:, :], EXP)

            eyet = cst.tile([128, 128], f16)
            nc.sync.dma_start(out=eyet[:, :], in_=eye[:, :])
            eyes1t = cst.tile([128, 128], f16)
            nc.gpsimd.dma_start(out=eyes1t[:, :], in_=eye_s1[:, :])

            # persistent E buffers: halo cols zeroed once (exp only ever
            # writes the 256 data cols) so L's halos are exactly ln(1)=0
            ebufs = []
            for bi in range(2):
                t = cst.tile([128, XF], f16, tag=f"e_{bi}")
                nc.vector.memset(
                    mk(t[:, 0:1], [[260, 2 * G], [258, 2], [1, 2]]), 0.0
                )
                ebufs.append(t)
            # persistent E2 buffers: partition 0 is never written by the
            # shift; zero it once so nothing reads uninitialized SBUF
            # (its outputs land in host-discarded rows anyway)
            e2bufs = []
            for bi in range(2):
                t = cst.tile([128, XF], f16, tag=f"e2_{bi}")
                nc.vector.memset(mk(t[0:1, 0:1], [[1, XF]]), 0.0)
                e2bufs.append(t)
            # persistent o buffers: halo cols zeroed once (stt writes only
            # data cols; halos are dumped but host-discarded)
            obufs = []
            for bi in range(2):
                t = cst.tile([128, OF], f16, tag=f"o_{bi}")
                nc.vector.memset(
                    mk(t[:, 0:1], [[260, 4 * 2 * G], [258, 2], [1, 2]]), 0.0
                )
                obufs.append(t)

            def front(grp):
                """Input load + Exp/Ln + shifted-E copies for one group."""
                n0 = grp * G
                # packed input: partition p <- rows {2p, 2p+1} as one
                # contiguous 512-elem (1KB fp16) run per instance
                X = iop.tile([128, G * 512], f16, tag="X")
                nc.sync.dma_start(
                    out=mk(X[:, 0:1], [[512, G], [1, 512]]),
                    in_=mkd(x[0, 0:128, :], [[512, 128], [XN, G], [1, 512]],
                            n0 * XN),
                )
                E = ebufs[grp % 2]
                nc.scalar.activation(
                    mk(E[:, 0:1], [[520, G], [260, 2], [1, 256]], 2),
                    mk(X[:, 0:1], [[512, G], [256, 2], [1, 256]]), EXP,
                )
                L = wp.tile([128, XF], f16, tag="L")
                nc.scalar.activation(L[:, :], E[:, :], LN, bias=1.0)
                # E2[p] = E[p-1]: the dy=-2 row shift, split by half-group
                # (so the first P-muls don't wait on the whole copy) and
                # 112+15 partitions for SDMA-engine spread
                E2 = e2bufs[grp % 2]
                for half in range(2):
                    off = half * (XF // 2)
                    nc.gpsimd.dma_start(
                        out=mk(E2[1:113, 0:1], [[1, XF // 2]], off),
                        in_=mk(E[0:112, 0:1], [[1, XF // 2]], off),
                    )
                    nc.gpsimd.dma_start(
                        out=mk(E2[113:128, 0:1], [[1, XF // 2]], off),
                        in_=mk(E[112:127, 0:1], [[1, XF // 2]], off),
                    )
                return E, E2, L

            def mid(grp, E, E2, L):
                """P products, ln(1+P), Lsum matmuls, combine into o."""
                # P[q,g,j,c] = E_c * E_y; q0..q2 need the row shift (in1 =
                # E2 at col bases 0,2,4), q3 is col-only (E at base 4).
                # q0..q2 on DVE, q3 on GpSimd.  P is bf16: e^{a+b} can
                # reach ~6e4, too close to fp16 max 65504.
                P = wp.tile([128, PF], bf16, tag="P")
                for g in range(G):
                    nc.vector.tensor_mul(
                        out=mk(P[:, 0:1], [[PQ, 3], [PJ, 2], [1, 256]], g * PG),
                        in0=mk(E[:, 0:1], [[0, 3], [XJ, 2], [1, 256]], g * XG + 2),
                        in1=mk(E2[:, 0:1], [[2, 3], [XJ, 2], [1, 256]], g * XG),
                    )
                    nc.gpsimd.tensor_mul(
                        out=mk(P[:, 0:1], [[PJ, 2], [1, 256]], 3 * PQ + g * PG),
                        in0=mk(E[:, 0:1], [[XJ, 2], [1, 256]], g * XG + 2),
                        in1=mk(E[:, 0:1], [[XJ, 2], [1, 256]], g * XG + 4),
                    )

                ln_t = wp.tile([128, PF], f16, tag="ln")
                o = obufs[grp % 2]

                # ln(1+P) in 4 big calls (q-pair h x half-group), each
                # unblocking its combines while the next runs
                for h in range(2):
                    for half in range(2):
                        nc.scalar.activation(
                            mk(ln_t[:, 0:1], [[PQ, 2], [1, 2 * PG]],
                               2 * h * PQ + half * 2 * PG),
                            mk(P[:, 0:1], [[PQ, 2], [1, 2 * PG]],
                               2 * h * PQ + half * 2 * PG), LN, bias=1.0,
                        )

                # L_y on the PE (one [128,512] matmul per quarter; PSUM out
                # is limited to one bank per matmul), grouped by weight so
                # the eye_s1->eye swaps are rare.  Per (h,g) one 2-bank
                # PSUM tile [q_even | q_odd]:
                #   h0: s1*L@0 -> b0 (q0), s1*L@2 -> b1 (q1)
                #   h1: s1*L@4 -> b0 (q2), eye*L@4 -> b1 (q3)
                # L_c joins on DVE in a second combine pass.
                for halfgs in ((0, 1), (2, 3)):
                    pss = {}
                    for g in halfgs:  # eye_s1 block
                        ps0 = psp.tile([128, 1024], f32, tag="ps")
                        pss[(0, g)] = ps0
                        nc.tensor.matmul(
                            ps0[:, 0:512], eyes1t[:, :],
                            mk(L[:, 0:1], [[XJ, 2], [1, 256]], g * XG),
                            start=True, stop=True,
                        )
                        nc.tensor.matmul(
                            ps0[:, 512:1024], eyes1t[:, :],
                            mk(L[:, 0:1], [[XJ, 2], [1, 256]], g * XG + 2),
                            start=True, stop=True,
                        )
                        ps1 = psp.tile([128, 1024], f32, tag="ps")
                        pss[(1, g)] = ps1
                        nc.tensor.matmul(
                            ps1[:, 0:512], eyes1t[:, :],
                            mk(L[:, 0:1], [[XJ, 2], [1, 256]], g * XG + 4),
                            start=True, stop=True,
                        )
                    for g in halfgs:  # eye block
                        nc.tensor.matmul(
                            pss[(1, g)][:, 512:1024], eyet[:, :],
                            mk(L[:, 0:1], [[XJ, 2], [1, 256]], g * XG + 4),
                            start=True, stop=True,
                        )
                    # o = (ln_t * -1) + L_y, then o += L_c (same L data for
                    # all four quarters: 0-stride q dim)
                    for g in halfgs:
                        for h in range(2):
                            osl = mk(o[:, 0:1], [[OQ, 2], [OJ, 2], [1, 256]],
                                     g * OG + 2 * h * OQ + 2)
                            nc.vector.scalar_tensor_tensor(
                                out=osl,
                                in0=mk(ln_t[:, 0:1], [[PQ, 2], [PJ, 2], [1, 256]],
                                       2 * h * PQ + g * PG),
                                scalar=-1.0,
                                in1=mk(pss[(h, g)][:, 0:1],
                                       [[512, 2], [256, 2], [1, 256]]),
                                op0=MULT, op1=ADD,
                            )
                            nc.vector.tensor_add(
                                out=osl,
                                in0=osl,
                                in1=mk(L[:, 0:1], [[0, 2], [XJ, 2], [1, 256]],
                                       g * XG + 2),
                            )
                return o

            def stores(grp, o, last):
                """One linear dump per instance: [128 partitions x 4160B]."""
                n0 = grp * G
                for g in range(G):
                    eng = nc.scalar if (last and g % 2 == 1) else nc.sync
                    eng.dma_start(
                        out=mkd(out[0, 0:1, 0:1], [[2080, 128], [1, 2080]],
                                (n0 + g) * ON),
                        in_=mk(o[:, 0:1], [[1, 2080]], g * OG),
                    )

            cur = front(0)
            for grp in range(NGRP):
                o = mid(grp, *cur)
                if grp + 1 < NGRP:
                    cur = front(grp + 1)
                stores(grp, o, last=(grp + 1 == NGRP))
    nc.compile()
    return nc


def _get_program():
    if "nc" not in _CACHE:
        _CACHE["nc"] = _build_program()
    return _CACHE["nc"]


def _in_maps(xf):
    """Per-core input dicts for run_bass_kernel_spmd from full [64,256,256]
    float32; input is downcast to fp16 on the host (x is ~N(0,1), |x|<6,
    so the cast costs <1e-3 absolute)."""
    x16 = xf.astype(np.float16)
    eye = np.eye(128).astype(np.float16)
    eye_s1 = np.eye(128, k=1).astype(np.float16)
    return [
        {
            "x": np.ascontiguousarray(x16[c * N_PER : (c + 1) * N_PER]),
            "eye": eye,
            "eye_s1": eye_s1,
        }
        for c in range(N_CORES)
    ]


def _assemble(outs):
    """Full [64, 8, 256, 256] f32 from the per-core quarter dumps.

    Each dump is [N_PER, 128, 2080] fp16, layout [n][p][q, j, cc]:
    value at (q, j, cc) = pairwise term of tap t(q) at row 2p+j, col cc-2.
    Quarters q0..q3 are taps (-2,-2), (-2,0), (-2,2), (0,2); tap 7-t is the
    mirror (values identical, shifted by (dy,dx)).  Out-of-range rows/cols
    (partition 0 of row-shifted quarters, halo cols) hold garbage the
    reference defines as 0 -- never copied, left as the zeros of np.zeros."""
    o = np.concatenate(outs, axis=0).astype(np.float32)  # [64, 128, 2080]
    o = (
        o.reshape(N_FULL, 128, 4, 2, 260)
        .transpose(0, 2, 1, 3, 4)
        .reshape(N_FULL, 4, 256, 260)
    )
    full = np.zeros((N_FULL, 8, H, W), np.float32)
    q0 = o[:, 0, 2:, 4:258]
    full[:, 0, 2:, 2:] = q0          # t0 = (-2,-2)
    full[:, 7, :254, :254] = q0      # t7 = (+2,+2)
    q1 = o[:, 1, 2:, 2:258]
    full[:, 1, 2:, :] = q1           # t1 = (-2, 0)
    full[:, 6, :254, :] = q1         # t6 = (+2, 0)
    q2 = o[:, 2, 2:, 2:256]
    full[:, 2, 2:, :254] = q2        # t2 = (-2,+2)
    full[:, 5, :254, 2:] = q2        # t5 = (+2,-2)
    q3 = o[:, 3, :, 2:256]
    full[:, 4, :, :254] = q3         # t4 = ( 0,+2)
    full[:, 3, :, 2:] = q3           # t3 = ( 0,-2)
    return full


def kernel(mask_logits, pairwise_size=3, pairwise_dilation=2, **_unused):
    assert int(pairwise_size) == 3 and int(pairwise_dilation) == 2
    from concourse.bass_utils import run_bass_kernel_spmd

    xf = np.ascontiguousarray(
        np.asarray(mask_logits, dtype=np.float32).reshape(N_FULL, H, W)
    )
    nc = _get_program()
    res = run_bass_kernel_spmd(nc, _in_maps(xf), core_ids=list(range(N_CORES)))
    return _assemble([res.results[c]["out"] for c in range(N_CORES)])


# revision 9
# speedup vs baseline: 1.1652x; 1.1652x over previous
"""Trainium2 Bass kernel: BoxSeg DynamicMaskHead compute_pairwise_term.

For each instance n and each of the 8 non-center taps (dy, dx) of a 3x3
dilation-2 stencil:

    out[n, t, h, w] = sp(x[h,w]) + sp(x[h+dy,w+dx]) - sp(x[h,w] + x[h+dy,w+dx])

with sp = softplus, computed as E = exp(x), L = ln(E + 1) and the tap term
ln(1 + E_c * E_y).  Mirror symmetry out[(dy,dx)][h,w] == out[(-dy,-dx)][h+dy,
w+dx] means only 4 of the 8 tap fields are unique; the device computes each
exactly once and dumps it linearly to DRAM, and the host assembles the full
[N, 8, H, W] tensor (mirror placement, boundary zeros, halo stripping, f32
upcast) -- pure data movement, no arithmetic.  That halves HBM write traffic
vs storing all 8 taps and lets every store be a full-width linear DMA
(4160B-per-partition descriptors at line rate instead of 512B row pieces).

Row-pair layout: partition p holds image rows {2p, 2p+1} (j = r % 2), so the
whole 256-row frame fits one tile and the dy=-2 row shift is a single
partition shift, materialized two ways: for E by an SBUF->SBUF partition-
shift DMA (E2), and for the softplus sum Lsum = L_c + L_y on the
TensorEngine by accumulating identity matmuls (eye and a k=1-shifted eye).
P = E_c * E_y runs on DVE (one quarter on GpSimd); ln(1+P) on ACT; the final
(ln_tap * -1) + Lsum on DVE reading PSUM.

Everything is fp16 (input downcast on host; P is bf16 since e^{a+b} can
reach ~6e4, too close to fp16 max).  ACT is the bottleneck engine
(~5 softplus-class evals per pixel are irreducible), so: the scalar engine
issues no DMAs, activations are merged into few large calls (the 352-cycle
per-call startup matters), and a dummy activation at t=0 overlaps the one
~2.7us ACT table load with the first input DMA.  Groups of G=4 instances
amortize fixed costs; 2 groups software-pipeline against each other.

Boundary handling is free: out-of-range rows/cols of each quarter (partition
0 of the row-shifted quarters, halo columns) compute garbage that the host
never reads -- it zero-fills those regions during assembly.

Sharding: data-parallel over N=64 -> 8 instances per core on 8 NeuronCores.
Self-contained: shapes hardcoded.
"""

import os

import numpy as np

N_CORES = 8
N_FULL = 64
N_PER = N_FULL // N_CORES  # 8 instances per core
H = W = 256
G = 4  # instances per group; 2 groups per core
NGRP = N_PER // G

# SBUF free-dim layouts (elements per partition); partition p = rows
# {2p, 2p+1}, j = r % 2, cc = image col + 2 (2-col halo each side).
# X: [g(G), jc(512)]
# E/L/E2: [g, j(2), cc(260)]
XG, XJ = 520, 260
XF = G * 520
# P/ln_t: [q(4), g, j, c(256)]
PQ, PG, PJ = G * 512, 512, 256
PF = 4 * G * 512
# o: [g, q, j, cc(260)]
OG, OQ, OJ = 2080, 520, 260
OF = G * 2080

XN = H * W  # x[n, r, c] instance stride
ON = 128 * 2080  # out[n, p, e] instance stride

_CACHE = {}


def _force_combined_act_table():
    """Make the table-load inserter see only the one set containing both Exp
    and Ln (all other sets emptied, positions preserved so act_func_set_id
    still indexes the real act_info.json).  Without this the inserter
    alternates between the exp- and ln-anchored sets: one 1.28us
    ACT_TABLE_LOAD per Exp<->Ln transition, which dominates the runtime."""
    import concourse.bacc as bacc
    import concourse.hw_specs as hw_specs
    import concourse.mybir as mybir

    real = dict(hw_specs.get_activation_tables("gen3"))
    target = None
    for name, fns in real.items():
        if (
            mybir.ActivationFunctionType.Exp in fns
            and mybir.ActivationFunctionType.Ln in fns
        ):
            target = name
            break
    assert target is not None, "no act table set with both Exp and Ln"
    patched = {
        name: (fns if name == target else set()) for name, fns in real.items()
    }
    bacc.get_activation_tables = lambda arch: patched
    hw_specs.get_activation_tables = lambda arch: patched


def _build_program():
    import concourse.bacc as bacc
    import concourse.mybir as mybir
    from concourse import tile

    if not os.environ.get("KERNEL_NO_ACT_PATCH"):
        _force_combined_act_table()

    f32 = mybir.dt.float32
    f16 = mybir.dt.float16
    bf16 = mybir.dt.bfloat16
    EXP = mybir.ActivationFunctionType.Exp
    LN = mybir.ActivationFunctionType.Ln
    ADD = mybir.AluOpType.add
    MULT = mybir.AluOpType.mult

    def mk(base, dims, off=0):
        """Rebuild the free dims of an AP: keep base's partition dim (ap[0]),
        replace the rest with `dims` ([step, count] in elements), and advance
        the offset by `off` elements."""
        c = base.copy()
        c.ap = mybir.VecI64Pair([list(c.ap[0])] + [list(d) for d in dims])
        c.offset = c.offset + off
        return c

    def mkd(base, dims, off=0):
        """Same for DRAM APs (no partition dim to preserve)."""
        c = base.copy()
        c.ap = mybir.VecI64Pair([list(d) for d in dims])
        c.offset = c.offset + off
        return c

    nc = bacc.Bacc(
        "TRN2",
        target_bir_lowering=False,
        debug=False,
        enable_asserts=False,
        num_devices=N_CORES,
    )
    x = nc.dram_tensor("x", [N_PER, H, W], f16, kind="ExternalInput").ap()
    out = nc.dram_tensor("out", [N_PER, 128, 2080], f16, kind="ExternalOutput").ap()
    eye = nc.dram_tensor("eye", [128, 128], f16, kind="ExternalInput").ap()
    eye_s1 = nc.dram_tensor("eye_s1", [128, 128], f16, kind="ExternalInput").ap()
    eye_w2 = nc.dram_tensor("eye_w2", [128, 128], f16, kind="ExternalInput").ap()

    with tile.TileContext(nc) as tc:
        with (
            tc.tile_pool(name="cst", bufs=1) as cst,
            tc.tile_pool(name="io", bufs=2) as iop,
            tc.tile_pool(name="wk", bufs=2) as wp,
            tc.tile_pool(name="ps", bufs=2, space="PSUM") as psp,
        ):
            # dummy activation with no data deps: pulls the one ACT table
            # load (~2.7us) to t=0, under the first input DMA
            dummy = cst.tile([128, 16], f16)
            nc.vector.memset(dummy[:, :], 0.0)
            nc.scalar.activation(dummy[:, :], dummy[:, :], EXP)

            eyet = cst.tile([128, 128], f16)
            nc.sync.dma_start(out=eyet[:, :], in_=eye[:, :])
            eyes1t = cst.tile([128, 128], f16)
            nc.sync.dma_start(out=eyes1t[:, :], in_=eye_s1[:, :])
            eyew2t = cst.tile([128, 128], f16)
            nc.sync.dma_start(out=eyew2t[:, :], in_=eye_w2[:, :])

            # persistent E buffers: halo cols zeroed once (exp only ever
            # writes the 256 data cols) so L's halos are exactly ln(1)=0
            ebufs = []
            for bi in range(2):
                t = cst.tile([128, XF], f16, tag=f"e_{bi}")
                nc.vector.memset(
                    mk(t[:, 0:1], [[260, 2 * G], [258, 2], [1, 2]]), 0.0
                )
                ebufs.append(t)
            # persistent E2 buffers: partition 0 is never written by the
            # shift; zero it once so nothing reads uninitialized SBUF
            # (its outputs land in host-discarded rows anyway)
            e2bufs = []
            for bi in range(2):
                t = cst.tile([128, XF], f16, tag=f"e2_{bi}")
                nc.vector.memset(mk(t[0:1, 0:1], [[1, XF]]), 0.0)
                e2bufs.append(t)
            # persistent o buffers: halo cols zeroed once (stt writes only
            # data cols; halos are dumped but host-discarded)
            obufs = []
            for bi in range(2):
                t = cst.tile([128, OF], f16, tag=f"o_{bi}")
                nc.vector.memset(
                    mk(t[:, 0:1], [[260, 4 * 2 * G], [258, 2], [1, 2]]), 0.0
                )
                obufs.append(t)

            def front(grp):
                """Input load + Exp/Ln + shifted-E copies for one group."""
                n0 = grp * G
                # packed input: partition p <- rows {2p, 2p+1} as one
                # contiguous 512-elem (1KB fp16) run per instance
                X = iop.tile([128, G * 512], f16, tag="X")
                nc.sync.dma_start(
                    out=mk(X[:, 0:1], [[512, G], [1, 512]]),
                    in_=mkd(x[0, 0:128, :], [[512, 128], [XN, G], [1, 512]],
                            n0 * XN),
                )
                E = ebufs[grp % 2]
                nc.scalar.activation(
                    mk(E[:, 0:1], [[520, G], [260, 2], [1, 256]], 2),
                    mk(X[:, 0:1], [[512, G], [256, 2], [1, 256]]), EXP,
                )
                L = wp.tile([128, XF], f16, tag="L")
                nc.scalar.activation(L[:, :], E[:, :], LN, bias=1.0)
                # E2[p] = E[p-1]: the dy=-2 row shift, 112+15 partitions for
                # SDMA-engine spread, on the HWDGE (sync) path -- SWDGE
                # (gpsimd) DMAs cost that engine ~1us of queue-drain each
                E2 = e2bufs[grp % 2]
                nc.sync.dma_start(
                    out=mk(E2[1:113, 0:1], [[1, XF]]),
                    in_=mk(E[0:112, 0:1], [[1, XF]]),
                )
                nc.sync.dma_start(
                    out=mk(E2[113:128, 0:1], [[1, XF]]),
                    in_=mk(E[112:127, 0:1], [[1, XF]]),
                )
                return E, E2, L

            def mid(grp, E, E2, L):
                """P products, ln(1+P), Lsum matmuls, combine into o."""
                # P[q,g,j,c] = E_c * E_y; q0..q2 need the row shift (in1 =
                # E2 at col bases 0,2,4), q3 is col-only (E at base 4).
                # One merged op per quarter across all G instances (DVE ops
                # pay ~400ns fixed each); q3 on the otherwise-idle GpSimd.
                # P is bf16: e^{a+b} can reach ~6e4, too close to fp16 max.
                P = wp.tile([128, PF], bf16, tag="P")
                for q in range(3):
                    nc.vector.tensor_mul(
                        out=mk(P[:, 0:1], [[PG, G], [PJ, 2], [1, 256]], q * PQ),
                        in0=mk(E[:, 0:1], [[XG, G], [XJ, 2], [1, 256]], 2),
                        in1=mk(E2[:, 0:1], [[XG, G], [XJ, 2], [1, 256]], 2 * q),
                    )
                nc.gpsimd.tensor_mul(
                    out=mk(P[:, 0:1], [[PG, G], [PJ, 2], [1, 256]], 3 * PQ),
                    in0=mk(E[:, 0:1], [[XG, G], [XJ, 2], [1, 256]], 2),
                    in1=mk(E[:, 0:1], [[XG, G], [XJ, 2], [1, 256]], 4),
                )

                ln_t = wp.tile([128, PF], f16, tag="ln")
                o = obufs[grp % 2]

                # ln(1+P) in 4 big calls, ordered (h0,A),(h1,A),(h0,B),
                # (h1,B) so half-group A's combines unblock after 2 calls
                for half in range(2):
                    for h in range(2):
                        nc.scalar.activation(
                            mk(ln_t[:, 0:1], [[PQ, 2], [1, 2 * PG]],
                               2 * h * PQ + half * 2 * PG),
                            mk(P[:, 0:1], [[PQ, 2], [1, 2 * PG]],
                               2 * h * PQ + half * 2 * PG), LN, bias=1.0,
                        )

                # Lsum = L_c + L_y on the PE (one [128,512] matmul per
                # psum bank), grouped by weight so swaps are rare.  Per g
                # one 4-bank PSUM tile [q0|q1|q2|q3]:
                #   eye:  Lc q0,q2,q3 (start) + Ly q3 = eye*L@4 (stop)
                #   s1:   Ly q0 = s1*L@0, Ly q2 = s1*L@4 (stop)
                #   w2:   q1 = (eye+s1)*L@2 in one matmul (start+stop)
                for halfgs in ((0, 1), (2, 3)):
                    pss = {}
                    for g in halfgs:  # eye block
                        ps = psp.tile([128, 2048], f32, tag="ps")
                        pss[g] = ps
                        for q in (0, 2, 3):
                            nc.tensor.matmul(
                                ps[:, q * 512:(q + 1) * 512], eyet[:, :],
                                mk(L[:, 0:1], [[XJ, 2], [1, 256]], g * XG + 2),
                                start=True, stop=False,
                            )
                        nc.tensor.matmul(
                            ps[:, 1536:2048], eyet[:, :],
                            mk(L[:, 0:1], [[XJ, 2], [1, 256]], g * XG + 4),
                            start=False, stop=True,
                        )
                    for g in halfgs:  # eye_s1 block
                        nc.tensor.matmul(
                            pss[g][:, 0:512], eyes1t[:, :],
                            mk(L[:, 0:1], [[XJ, 2], [1, 256]], g * XG),
                            start=False, stop=True,
                        )
                        nc.tensor.matmul(
                            pss[g][:, 1024:1536], eyes1t[:, :],
                            mk(L[:, 0:1], [[XJ, 2], [1, 256]], g * XG + 4),
                            start=False, stop=True,
                        )
                    for g in halfgs:  # eye+eye_s1 block
                        nc.tensor.matmul(
                            pss[g][:, 512:1024], eyew2t[:, :],
                            mk(L[:, 0:1], [[XJ, 2], [1, 256]], g * XG + 2),
                            start=True, stop=True,
                        )
                    # o = (ln_t * -1) + Lsum, one merged op per instance
                    for g in halfgs:
                        nc.vector.scalar_tensor_tensor(
                            out=mk(o[:, 0:1], [[OQ, 4], [OJ, 2], [1, 256]],
                                   g * OG + 2),
                            in0=mk(ln_t[:, 0:1], [[PQ, 4], [PJ, 2], [1, 256]],
                                   g * PG),
                            scalar=-1.0,
                            in1=mk(pss[g][:, 0:1],
                                   [[512, 4], [256, 2], [1, 256]]),
                            op0=MULT, op1=ADD,
                        )
                return o

            def stores(grp, o, last):
                """One linear dump per instance: [128 partitions x 4160B]."""
                n0 = grp * G
                for g in range(G):
                    eng = nc.scalar if (last and g % 2 == 1) else nc.sync
                    eng.dma_start(
                        out=mkd(out[0, 0:1, 0:1], [[2080, 128], [1, 2080]],
                                (n0 + g) * ON),
                        in_=mk(o[:, 0:1], [[1, 2080]], g * OG),
                    )

            cur = front(0)
            for grp in range(NGRP):
                o = mid(grp, *cur)
                if grp + 1 < NGRP:
                    cur = front(grp + 1)
                stores(grp, o, last=(grp + 1 == NGRP))
    nc.compile()
    return nc


def _get_program():
    if "nc" not in _CACHE:
        _CACHE["nc"] = _build_program()
    return _CACHE["nc"]


def _in_maps(xf):
    """Per-core input dicts for run_bass_kernel_spmd from full [64,256,256]
    float32; input is downcast to fp16 on the host (x is ~N(0,1), |x|<6,
    so the cast costs <1e-3 absolute)."""
    x16 = xf.astype(np.float16)
    eye = np.eye(128).astype(np.float16)
    eye_s1 = np.eye(128, k=1).astype(np.float16)
    eye_w2 = (np.eye(128) + np.eye(128, k=1)).astype(np.float16)
    return [
        {
            "x": np.ascontiguousarray(x16[c * N_PER : (c + 1) * N_PER]),
            "eye": eye,
            "eye_s1": eye_s1,
            "eye_w2": eye_w2,
        }
        for c in range(N_CORES)
    ]


def _assemble(outs):
    """Full [64, 8, 256, 256] f32 from the per-core quarter dumps.

    Each dump is [N_PER, 128, 2080] fp16, layout [n][p][q, j, cc]:
    value at (q, j, cc) = pairwise term of tap t(q) at row 2p+j, col cc-2.
    Quarters q0..q3 are taps (-2,-2), (-2,0), (-2,2), (0,2); tap 7-t is the
    mirror (values identical, shifted by (dy,dx)).  Out-of-range rows/cols
    (partition 0 of row-shifted quarters, halo cols) hold garbage the
    reference defines as 0 -- never copied, left as the zeros of np.zeros."""
    o = np.concatenate(outs, axis=0).astype(np.float32)  # [64, 128, 2080]
    o = (
        o.reshape(N_FULL, 128, 4, 2, 260)
        .transpose(0, 2, 1, 3, 4)
        .reshape(N_FULL, 4, 256, 260)
    )
    full = np.zeros((N_FULL, 8, H, W), np.float32)
    q0 = o[:, 0, 2:, 4:258]
    full[:, 0, 2:, 2:] = q0          # t0 = (-2,-2)
    full[:, 7, :254, :254] = q0      # t7 = (+2,+2)
    q1 = o[:, 1, 2:, 2:258]
    full[:, 1, 2:, :] = q1           # t1 = (-2, 0)
    full[:, 6, :254, :] = q1         # t6 = (+2, 0)
    q2 = o[:, 2, 2:, 2:256]
    full[:, 2, 2:, :254] = q2        # t2 = (-2,+2)
    full[:, 5, :254, 2:] = q2        # t5 = (+2,-2)
    q3 = o[:, 3, :, 2:256]
    full[:, 4, :, :254] = q3         # t4 = ( 0,+2)
    full[:, 3, :, 2:] = q3           # t3 = ( 0,-2)
    return full


def kernel(mask_logits, pairwise_size=3, pairwise_dilation=2, **_unused):
    assert int(pairwise_size) == 3 and int(pairwise_dilation) == 2
    from concourse.bass_utils import run_bass_kernel_spmd

    xf = np.ascontiguousarray(
        np.asarray(mask_logits, dtype=np.float32).reshape(N_FULL, H, W)
    )
    nc = _get_program()
    res = run_bass_kernel_spmd(nc, _in_maps(xf), core_ids=list(range(N_CORES)))
    return _assemble([res.results[c]["out"] for c in range(N_CORES)])
